# revision 22
# baseline (speedup 1.0000x reference)
"""Multi-head attention (B=4, S=2048, E=1024, H=16, D=64) on 8 trn2 NeuronCores.

Sharding: core c handles batch b=c//2 and query-row half h=c%2 (rows
h*1024:(h+1)*1024). Each core computes all 16 heads for its 1024 query rows
against the full 2048 keys/values of its batch, then the full output
projection for those rows — so no cross-core collectives are needed; the host
just concatenates the 8 disjoint row-blocks.

Per-core dataflow (everything f32; matmuls in float32r = fast 4-byte mode):
  1. PE-transpose inputs to feature-major: xvT/xkT/xqT [e, t|s].
  2. Per-head projections QT/KT [d, t] (pairs of heads packed to 128
     partitions) and V [t, d] (8-head groups packed to 512-wide psum).
  3. scoresT[t, s] = KT.T @ QT per (head, t-chunk); exp via ScalarE with
     scale=1/8 (softmax max-subtraction skipped: |scores/8| < ~4 so exp is
     safe in f32); ctx^T accumulated as V'.T @ expT where V' carries an
     extra ones column that yields the softmax denominator for free.
  4. Normalize via reciprocal + PE outer-product broadcast, spill ctx^T
     [i, s] to DRAM.
  5. y[s, o] = ctx^T.T @ fc_w^T + b, with the bias injected as a K=1
     ones-outer-product matmul that initializes the PSUM accumulator.
"""

import numpy as np

import concourse.bass as bass
import concourse.mybir as mybir
import concourse.tile as tile
from concourse import bacc
from concourse.bass_utils import run_bass_kernel_spmd

B, S, E, H, D = 4, 2048, 1024, 16, 64
SL = S // 2          # local query rows per core
EC = E // 128        # e-chunks
TC = S // 128        # t-chunks (keys)
R = mybir.dt.float32r
F = mybir.dt.float32
Exp = mybir.ActivationFunctionType.Exp

_CACHE = {}


def build_nc():
    nc = bacc.Bacc("TRN2", target_bir_lowering=False, debug=False, num_devices=8)

    xq = nc.dram_tensor("xq", [SL, E], R, kind="ExternalInput")
    xk = nc.dram_tensor("xk", [S, E], R, kind="ExternalInput")
    xv = nc.dram_tensor("xv", [S, E], R, kind="ExternalInput")
    wq = nc.dram_tensor("wq", [H, E, D], R, kind="ExternalInput")
    wk = nc.dram_tensor("wk", [H, E, D], R, kind="ExternalInput")
    wv = nc.dram_tensor("wv", [H, E, D], R, kind="ExternalInput")
    fcw = nc.dram_tensor("fcw", [E, E], R, kind="ExternalInput")
    fcb = nc.dram_tensor("fcb", [E], R, kind="ExternalInput")
    ident = nc.dram_tensor("ident", [128, 128], R, kind="ExternalInput")
    ones2 = nc.dram_tensor("ones2", [128, 128], R, kind="ExternalInput")
    y = nc.dram_tensor("y", [SL, E], R, kind="ExternalOutput")

    ctx_spill = nc.dram_tensor("ctx_spill", [E, SL], R)

    with tile.TileContext(nc) as tc:
        from contextlib import ExitStack

        with ExitStack() as top:
            const = top.enter_context(tc.tile_pool(name="const", bufs=1))
            ident_sb = const.tile([128, 128], R, name="ident_sb")
            nc.sync.dma_start(out=ident_sb[:], in_=ident[:])
            ones2_sb = const.tile([128, 128], R, name="ones2_sb")
            nc.sync.dma_start(out=ones2_sb[:], in_=ones2[:])
            ones_sb = ones2_sb[0:1, :]
            fcb_sb = const.tile([1, E], R, name="fcb_sb")
            nc.sync.dma_start(
                out=fcb_sb[:], in_=fcb[:].rearrange("(a o) -> a o", a=1)
            )

            # persistent tensors for the proj+attention phases; pools are
            # created at their phase start (allocation happens at creation)
            mid = ExitStack()

            def load_w(pool, wt, h0, nh, name):
                t = pool.tile([128, EC, nh * D], R, name=name)
                tv = t[:].rearrange("p ec (h d) -> p ec h d", h=nh)
                for j in range(nh):
                    nc.sync.dma_start(
                        out=tv[:, :, j, :],
                        in_=wt[h0 + j].rearrange("(ec p) d -> p ec d", p=128),
                    )
                return t

            # ================= projection phases =================
            with ExitStack() as proj:
                natp = proj.enter_context(tc.tile_pool(name="nat", bufs=3, side="right"))
                xtp = proj.enter_context(tc.tile_pool(name="xt", bufs=1, side="right"))
                pst = proj.enter_context(
                    tc.tile_pool(name="pst", bufs=2, space="PSUM")
                )
                psp = proj.enter_context(
                    tc.tile_pool(name="psp", bufs=3, space="PSUM")
                )

                def transpose_chunk(src_dram, row0, xt_tile, sc4):
                    """Load 128 rows of [*, E] source, PE-transpose into
                    xt_tile[:, :, sc4*128:(sc4+1)*128]."""
                    nat = natp.tile([128, E], R, name="nat")
                    nc.sync.dma_start(out=nat[:], in_=src_dram[row0 : row0 + 128, :])
                    pbig = pst.tile([128, E], R, name="pbig")
                    for ec in range(EC):
                        nc.tensor.transpose(
                            pbig[:, ec * 128 : (ec + 1) * 128],
                            nat[:, ec * 128 : (ec + 1) * 128],
                            ident_sb[:],
                        )
                    nc.vector.tensor_copy(
                        xt_tile[:, :, sc4 * 128 : (sc4 + 1) * 128],
                        pbig[:].rearrange("p (ec t) -> p ec t", ec=EC),
                    )

                # ---- Q phase ----
                wqpool = proj.enter_context(tc.tile_pool(name="wqld", bufs=2, side="right"))
                qtp = mid.enter_context(tc.tile_pool(name="qtp", bufs=1))
                QT = [qtp.tile([128, SL], R, name=f"qt{p}") for p in range(8)]
                xqt = xtp.tile([128, EC, 512], R, name="xt")
                for sh in range(2):
                    for sc4 in range(4):
                        transpose_chunk(xq, sh * 512 + sc4 * 128, xqt, sc4)
                    for p in range(8):
                        wq_p = load_w(wqpool, wq, 2 * p, 2, "wq_p")
                        pq = psp.tile([128, 512], F, name="pp")
                        for ec in range(EC):
                            nc.tensor.matmul(
                                pq[:],
                                wq_p[:, ec, :],
                                xqt[:, ec, :],
                                start=(ec == 0),
                                stop=(ec == EC - 1),
                            )
                        nc.vector.tensor_copy(
                            QT[p][:, sh * 512 : (sh + 1) * 512], pq[:]
                        )

                # ---- V phase ----
                vpool = mid.enter_context(tc.tile_pool(name="vs", bufs=1))
                # [p(t%128), tc, 8 heads x 65]; col h*65+0 = ones column,
            # cols h*65+1..h*65+64 = V_h — so the softmax denominator row
            # lands on PSUM partition 0 (keeps DVE ops partition-aligned)
                VS = [vpool.tile([128, TC, 8 * 65], R, name=f"vs{g}")
                      for g in range(2)]
                wvstack = ExitStack()
                wvpool = wvstack.enter_context(tc.tile_pool(name="wvld", bufs=1, side="right"))
                wv_g = [load_w(wvpool, wv, g * 8, 8, f"wv{g}") for g in range(2)]
                for g in range(2):
                    ones_view = VS[g][:].rearrange(
                        "p t (h c) -> p t h c", h=8
                    )[:, :, :, 0]
                    nc.vector.tensor_copy(
                        ones_view,
                        ones2_sb[:, 0:TC * 8].rearrange(
                            "p (a b) -> p a b", a=TC
                        ),
                    )
                xvt = xtp.tile([128, EC, 512], R, name="xt")
                for tt in range(4):
                    for sc4 in range(4):
                        transpose_chunk(xv, tt * 512 + sc4 * 128, xvt, sc4)
                    for g in range(2):
                        for tc4 in range(4):
                            t_abs = tt * 4 + tc4
                            pv = psp.tile([128, 512], F, name="pp")
                            for ec in range(EC):
                                nc.tensor.matmul(
                                    pv[:],
                                    xvt[:, ec, tc4 * 128 : (tc4 + 1) * 128],
                                    wv_g[g][:, ec, :],
                                    start=(ec == 0),
                                    stop=(ec == EC - 1),
                                )
                            nc.vector.tensor_copy(
                                VS[g][:, t_abs, :]
                                .rearrange("p (h c) -> p h c", h=8)[:, :, 1:65],
                                pv[:].rearrange("p (h d) -> p h d", h=8),
                            )

                # ---- K phase ----
                wvstack.close()
                ktp = mid.enter_context(tc.tile_pool(name="ktp", bufs=1))
                KT = [ktp.tile([128, S], R, name=f"kt{p}") for p in range(8)]
                xkt = xtp.tile([128, EC, 512], R, name="xt")
                for tt in range(4):
                    for sc4 in range(4):
                        transpose_chunk(xk, tt * 512 + sc4 * 128, xkt, sc4)
                    for p in range(8):
                        wk_p = load_w(wqpool, wk, 2 * p, 2, "wq_p")
                        pk = psp.tile([128, 512], F, name="pp")
                        for ec in range(EC):
                            nc.tensor.matmul(
                                pk[:],
                                wk_p[:, ec, :],
                                xkt[:, ec, :],
                                start=(ec == 0),
                                stop=(ec == EC - 1),
                            )
                        nc.vector.tensor_copy(
                            KT[p][:, tt * 512 : (tt + 1) * 512], pk[:]
                        )

            # ================= attention =================
            with ExitStack() as attn:
                scp = attn.enter_context(
                    tc.tile_pool(name="scp", bufs=2, space="PSUM")
                )
                ctxp = attn.enter_context(
                    tc.tile_pool(name="ctxp", bufs=1, space="PSUM")
                )
                bcp = attn.enter_context(
                    tc.tile_pool(name="bcp", bufs=2, space="PSUM")
                )
                expp = attn.enter_context(tc.tile_pool(name="expp", bufs=2))
                rcp = attn.enter_context(tc.tile_pool(name="rcp", bufs=2))
                bcs = attn.enter_context(tc.tile_pool(name="bcs", bufs=2))
                ctp = attn.enter_context(tc.tile_pool(name="ctp", bufs=2))

                for h in range(H):
                    g, hh = h // 8, h % 8
                    p, off = h // 2, (h % 2) * 64
                    pctx = ctxp.tile([65, SL], F, name="pctx")
                    for tcn in range(TC):
                        psc = scp.tile([128, SL], F, name="psc")
                        for sh in range(2):
                            nc.tensor.matmul(
                                psc[:, sh * 512 : (sh + 1) * 512],
                                KT[p][off : off + 64, tcn * 128 : (tcn + 1) * 128],
                                QT[p][off : off + 64, sh * 512 : (sh + 1) * 512],
                                start=True,
                                stop=True,
                            )
                        et = expp.tile([128, SL], R, name="et")
                        nc.scalar.activation(et[:], psc[:], Exp, scale=0.125)
                        for sh in range(2):
                            nc.tensor.matmul(
                                pctx[:, sh * 512 : (sh + 1) * 512],
                                VS[g][:, tcn, hh * 65 : hh * 65 + 65],
                                et[:, sh * 512 : (sh + 1) * 512],
                                start=(tcn == 0),
                                stop=(tcn == TC - 1),
                            )
                    # reciprocal_approx_fast with a float32r-typed output so
                    # the result is legal as an f32r matmul operand (the
                    # public wrapper asserts fp32, but R is bit-identical)
                    from concourse.dve_ops import (
                        RECIP_APPROX_FAST_CONSTS as _RC,
                        RECIPROCAL_APPROX_FAST as _RF,
                    )

                    rc = rcp.tile([1, SL], R, name="rc")
                    nc.vector._custom_dve(
                        _RF,
                        out=rc[:],
                        in0=pctx[0:1, :],
                        s0=_RC["s0"],
                        s1=_RC["s1"],
                        imm2=_RC["imm2"],
                    )
                    bc_sb = bcs.tile([65, SL], F, name="bc_sb")
                    for sh in range(2):
                        pbc = bcp.tile([65, 512], F, name="pbc")
                        nc.tensor.matmul(
                            pbc[:],
                            ones_sb[:, 0:65],
                            rc[:, sh * 512 : (sh + 1) * 512],
                            start=True,
                            stop=True,
                        )
                        nc.vector.tensor_copy(
                            bc_sb[:, sh * 512 : (sh + 1) * 512], pbc[:]
                        )
                    ctmp = ctp.tile([65, SL], R, name="ctmp")
                    nc.vector.tensor_tensor(
                        ctmp[:],
                        pctx[:],
                        bc_sb[:],
                        op=mybir.AluOpType.mult,
                    )
                    nc.sync.dma_start(
                        out=ctx_spill[h * 64 : (h + 1) * 64, :], in_=ctmp[1:65, :]
                    )

            # ================= output projection =================
            mid.close()
            with ExitStack() as fcs:
                clp = fcs.enter_context(tc.tile_pool(name="clp", bufs=1))
                fwp = fcs.enter_context(tc.tile_pool(name="fwp", bufs=1))
                fnp = fcs.enter_context(tc.tile_pool(name="fnp", bufs=2))
                ysb = fcs.enter_context(tc.tile_pool(name="ysb", bufs=2))
                psf = fcs.enter_context(
                    tc.tile_pool(name="psf", bufs=2, space="PSUM")
                )
                psy = fcs.enter_context(
                    tc.tile_pool(name="psy", bufs=4, space="PSUM")
                )

                fcwT = fwp.tile([128, EC, E], R, name="fcwT")
                for oc in range(EC):
                    fnat = fnp.tile([128, E], R, name="fnat")
                    nc.sync.dma_start(
                        out=fnat[:], in_=fcw[oc * 128 : (oc + 1) * 128, :]
                    )
                    for icg in range(2):
                        pfw = psf.tile([128, 512], R, name="pfw")
                        for j in range(4):
                            ic = icg * 4 + j
                            nc.tensor.transpose(
                                pfw[:, j * 128 : (j + 1) * 128],
                                fnat[:, ic * 128 : (ic + 1) * 128],
                                ident_sb[:],
                            )
                        nc.vector.tensor_copy(
                            fcwT[:, icg * 4 : icg * 4 + 4, oc * 128 : (oc + 1) * 128],
                            pfw[:].rearrange("p (ic o) -> p ic o", ic=4),
                        )

                ctx_ld = []
                for ic in range(EC):
                    t = clp.tile([128, SL], R, name=f"cl{ic}")
                    nc.sync.dma_start(
                        out=t[:], in_=ctx_spill[ic * 128 : (ic + 1) * 128, :]
                    )
                    ctx_ld.append(t)

                for sc in range(SL // 128):
                    yt = ysb.tile([128, E], R, name="yt")
                    for oh in range(2):
                        py = psy.tile([128, 512], F, name="py")
                        nc.tensor.matmul(
                            py[:],
                            ones_sb[:, :],
                            fcb_sb[0:1, oh * 512 : (oh + 1) * 512],
                            start=True,
                            stop=False,
                        )
                        for ic in range(EC):
                            nc.tensor.matmul(
                                py[:],
                                ctx_ld[ic][:, sc * 128 : (sc + 1) * 128],
                                fcwT[:, ic, oh * 512 : (oh + 1) * 512],
                                start=False,
                                stop=(ic == EC - 1),
                            )
                        nc.vector.tensor_copy(
                            yt[:, oh * 512 : (oh + 1) * 512].bitcast(F), py[:]
                        )
                    nc.sync.dma_start(
                        out=y[sc * 128 : (sc + 1) * 128, :], in_=yt[:]
                    )

    nc.compile()
    return nc


def _get_nc():
    if "nc" not in _CACHE:
        _CACHE["nc"] = build_nc()
    return _CACHE["nc"]


def kernel(**inputs):
    q = np.asarray(inputs["query"], dtype=np.float32)
    k = np.asarray(inputs["key"], dtype=np.float32)
    v = np.asarray(inputs["value"], dtype=np.float32)
    wq = np.ascontiguousarray(np.asarray(inputs["wq"], dtype=np.float32))
    wk = np.ascontiguousarray(np.asarray(inputs["wk"], dtype=np.float32))
    wv = np.ascontiguousarray(np.asarray(inputs["wv"], dtype=np.float32))
    fcw = np.ascontiguousarray(np.asarray(inputs["fc_w"], dtype=np.float32))
    fcb = np.ascontiguousarray(np.asarray(inputs["fc_b"], dtype=np.float32))
    ident = np.eye(128, dtype=np.float32)
    ones2 = np.ones((128, 128), dtype=np.float32)

    nc = _get_nc()
    in_maps = []
    for c in range(8):
        b, half = c // 2, c % 2
        in_maps.append(
            {
                "xq": np.ascontiguousarray(q[b, half * SL : (half + 1) * SL, :]),
                "xk": np.ascontiguousarray(k[b]),
                "xv": np.ascontiguousarray(v[b]),
                "wq": wq,
                "wk": wk,
                "wv": wv,
                "fcw": fcw,
                "fcb": fcb,
                "ident": ident,
                "ones2": ones2,
            }
        )
    r = run_bass_kernel_spmd(nc, in_maps, core_ids=list(range(8)))
    out = np.empty((B, S, E), dtype=np.float32)
    for c in range(8):
        b, half = c // 2, c % 2
        out[b, half * SL : (half + 1) * SL, :] = r.results[c]["y"]
    return out
